# revision 2
# baseline (speedup 1.0000x reference)
"""Trainium2 Bass kernel for the CWLNFace margin-softmax loss head.

Raw-bass hand-scheduled pipeline (the Tile framework's per-matmul
semaphore traffic costs ~70 ns/matmul = ~20 us/run, measured):

  Device computes outT[c, b] = emb_b . wpre_c  (wpre = kernel scaled by
  S/||col|| on the host, bf16), stored bf16; the u0 half is written
  unclipped by ACT (bf16 range is ample), the u1 half clipped by DVE;
  host clips everything, patches the B label entries exactly,
  transposes.

Per-core steady state (C sharded 8 x 8960 = 70 C-tiles, 35 macros):
  PE   8 x 512-row matmuls/macro        ~1.69 us/macro  <- critical
  ACT  weight DMA issue + u0 copy       ~1.4 us engine
  DVE  u1 clip+convert                  ~1.05 us
  SP   out DMA issue                    ~0.6 us

Synchronization: engine-incremented sems (g0/g1 from PE group-final
matmuls, c0/c1 from the consumers) are in program order, so shared
count-based waits are exact.  DMA completions on a ring can reorder,
so every DMA-completion wait uses a per-buffer-slot semaphore with at
most one in-flight DMA per slot.  NB_W and NB_O divide NM so slot
schedules align across the hardware reps loop.
"""

import math
import numpy as np

B = 512
EMB = 512
C = 70722
NCORES = 8
CSH = 8960
NT = CSH // 128
NM = NT // 2        # 35 macro-tiles of 2 C-tiles
S = 64.0
EPS = 1e-3
MARGIN = 0.4
H = 0.333
CLIP_HI = S * (1.0 - EPS)

NB_W = 7            # weight buffers; must divide NM
NB_P = 4            # PSUM tiles (2 banks each = all 8 banks)
NB_O = 5            # output staging buffers; must divide NM

_CACHE = {}


def _build_nc(reps=1):
    from contextlib import ExitStack

    from concourse import bacc, mybir

    f32 = mybir.dt.float32
    bf16 = mybir.dt.bfloat16
    AF = mybir.ActivationFunctionType
    OP = mybir.AluOpType

    nc = bacc.Bacc(
        "TRN2",
        target_bir_lowering=False,
        debug=False,
        enable_asserts=False,
    )

    embT = nc.dram_tensor("embT", [4, 128, B], bf16, kind="ExternalInput").ap()
    ksh = nc.dram_tensor(
        "ksh", [NM, 128, 2, 4, 128], bf16, kind="ExternalInput"
    ).ap()
    out = nc.dram_tensor(
        "out", [NM, 128, 2, B], bf16, kind="ExternalOutput"
    ).ap()

    with ExitStack() as ctx:
        E = ctx.enter_context
        block = E(nc.Block())
        esem = E(nc.semaphore("esem"))
        wsem = [E(nc.semaphore(f"wsem{j}")) for j in range(NB_W)]
        ssem = [E(nc.semaphore(f"ssem{j}")) for j in range(NB_O)]
        g0 = E(nc.semaphore("g0"))
        g1 = E(nc.semaphore("g1"))
        c0 = E(nc.semaphore("c0"))
        c1 = E(nc.semaphore("c1"))
        emb_sb = E(nc.sbuf_tensor("emb_sb", [128, 4, B], bf16))
        w_sb = [
            E(nc.sbuf_tensor(f"w_sb{j}", [128, 2, 4, 128], bf16))
            for j in range(NB_W)
        ]
        o_sb = [
            E(nc.sbuf_tensor(f"o_sb{j}", [128, 2, B], bf16))
            for j in range(NB_O)
        ]
        pc = [
            E(nc.psum_tensor(f"pc{j}", [128, 2, B], f32)) for j in range(NB_P)
        ]

        class Ctr:
            """Wait-value counter: python int at reps==1, else a register."""

            def __init__(self, eng, name, init, delta):
                self.delta = delta
                if reps == 1:
                    self.v = init
                else:
                    self.r = E(eng.register(name))
                    eng.reg_mov(self.r, init)
                    self.eng = eng

            def val(self):
                return self.v if reps == 1 else self.r

            def bump(self):
                if reps == 1:
                    self.v += self.delta
                else:
                    self.eng.reg_add(self.r, self.r, self.delta)

        # Timing builds unroll UNROLL reps per hardware-loop iteration:
        # 140 macros is a multiple of NB_P/NB_O/NB_W/NM, so buffer-slot
        # schedules stay aligned across iterations (a bare 35-macro body
        # misaligns the 4-deep PSUM ring at the rep boundary).
        UNROLL = 4
        assert reps == 1 or reps % UNROLL == 0, reps

        def macro_loop(eng, emit):
            if reps == 1:
                for mu in range(NM):
                    emit(mu)
            else:
                with eng.Fori(0, reps // UNROLL):
                    for mu in range(NM * UNROLL):
                        emit(mu)

        @block.sync
        def _(sp):
            sp.dma_start(
                emb_sb[:], embT.rearrange("c p b -> p c b")
            ).then_inc(esem, 16)
            rc0 = Ctr(sp, "sp_c0", NB_P + 1, 1)
            rc1 = Ctr(sp, "sp_c1", NB_P + 1, 1)

            def emit(mu):
                sp.wait_ge(c0, rc0.val())
                sp.wait_ge(c1, rc1.val())
                sp.dma_start(out[mu % NM], o_sb[mu % NB_O][:]).then_inc(
                    ssem[mu % NB_O], 16
                )
                rc0.bump()
                rc1.bump()

            macro_loop(sp, emit)
            for j in range(NB_O):
                sp.wait_ge(ssem[j], 16 * (NM // NB_O) * reps)

        @block.scalar
        def _(act):
            for j in range(NB_W):
                act.dma_start(w_sb[j][:], ksh[j]).then_inc(wsem[j], 16)
            rg1 = Ctr(act, "act_g1", 1, 1)
            rg0 = Ctr(act, "act_g0", 1, 1)
            rs = [Ctr(act, f"act_s{j}", 0, 16) for j in range(NB_O)]

            def emit(mu):
                act.wait_ge(g1, rg1.val())
                if reps > 1 or mu < NM - NB_W:
                    act.dma_start(
                        w_sb[mu % NB_W][:], ksh[(mu + NB_W) % NM]
                    ).then_inc(wsem[mu % NB_W], 16)
                act.wait_ge(g0, rg0.val())
                act.wait_ge(ssem[mu % NB_O], rs[mu % NB_O].val())
                act.activation(
                    o_sb[mu % NB_O][:, 0, :], pc[mu % NB_P][:, 0, :], AF.Copy
                ).then_inc(c0, 1)
                rg1.bump()
                rg0.bump()
                rs[mu % NB_O].bump()

            macro_loop(act, emit)

        @block.vector
        def _(dve):
            rg1 = Ctr(dve, "dve_g1", 1, 1)
            rs = [Ctr(dve, f"dve_s{j}", 0, 16) for j in range(NB_O)]

            def emit(mu):
                dve.wait_ge(g1, rg1.val())
                dve.wait_ge(ssem[mu % NB_O], rs[mu % NB_O].val())
                dve.tensor_scalar(
                    o_sb[mu % NB_O][:, 1, :], pc[mu % NB_P][:, 1, :],
                    CLIP_HI, -CLIP_HI, OP.min, OP.max,
                ).then_inc(c1, 1)
                rg1.bump()
                rs[mu % NB_O].bump()

            macro_loop(dve, emit)

        @block.tensor
        def _(pe):
            pe.sem_inc(c0, NB_P)
            pe.sem_inc(c1, NB_P)
            pe.wait_ge(esem, 16)
            rw = [Ctr(pe, f"pe_w{j}", 16, 16) for j in range(NB_W)]
            rc0 = Ctr(pe, "pe_c0", 1, 1)
            rc1 = Ctr(pe, "pe_c1", 1, 1)

            def emit(mu):
                pe.wait_ge(wsem[mu % NB_W], rw[mu % NB_W].val())
                pe.wait_ge(c0, rc0.val())
                pe.wait_ge(c1, rc1.val())
                p = pc[mu % NB_P]
                w = w_sb[mu % NB_W]
                for u in range(2):
                    for c in range(4):
                        mm = pe.matmul(
                            p[:, u, :],
                            lhsT=w[:, u, c, :],
                            rhs=emb_sb[:, c, :],
                            start=(c == 0),
                            stop=(c == 3),
                        )
                        if c == 3:
                            mm.then_inc(g0 if u == 0 else g1, 1)
                rw[mu % NB_W].bump()
                rc0.bump()
                rc1.bump()

            macro_loop(pe, emit)

    nc.compile()
    return nc


def _get_nc():
    if "nc" not in _CACHE:
        _CACHE["nc"] = _build_nc()
    return _CACHE["nc"]


def make_shards(kfull):
    """Pre-scaled bf16 weight shards, macro-major [NM, 128, 2, 4, 128]."""
    import ml_dtypes

    bf16 = np.dtype(ml_dtypes.bfloat16)
    kfull = np.asarray(kfull, dtype=np.float32)
    scale = S / np.linalg.norm(kfull.astype(np.float64), axis=0)
    wpre = (kfull * scale.astype(np.float32)[None, :]).astype(bf16)
    shards = []
    for i in range(NCORES):
        lo, hi = i * CSH, (i + 1) * CSH
        if hi <= C:
            shard = wpre[:, lo:hi]
        else:
            shard = np.zeros((EMB, CSH), dtype=bf16)
            shard[:, : C - lo] = wpre[:, lo:C]
        tiled = shard.reshape(4, 128, NM, 2, 128).transpose(2, 1, 3, 0, 4)
        shards.append(np.ascontiguousarray(tiled))
    return shards


def make_embT(embbedings):
    import ml_dtypes

    embT = np.asarray(embbedings, dtype=np.float32).T.astype(ml_dtypes.bfloat16)
    return np.ascontiguousarray(embT.reshape(4, 128, B))


def run_device(embbedings, kernel, trace=False):
    from concourse.bass_utils import run_bass_kernel_spmd

    nc = _get_nc()
    embT = make_embT(embbedings)
    in_maps = [{"embT": embT, "ksh": shard} for shard in make_shards(kernel)]

    res = run_bass_kernel_spmd(nc, in_maps, core_ids=list(range(NCORES)), trace=trace)
    parts = [
        np.asarray(r["out"]).transpose(0, 2, 1, 3).reshape(CSH, B)
        for r in res.results
    ]
    outT = np.concatenate(parts, axis=0)[:C].astype(np.float32)
    np.clip(outT, -S * (1 - EPS), S * (1 - EPS), out=outT)
    return outT, res


def kernel(embbedings, norms, label, class_sample_num_, kernel):
    outT, _ = run_device(embbedings, kernel)

    # ---- host margin fix-up (touches exactly B entries) ----
    norms = np.asarray(norms, dtype=np.float32)
    csn = np.asarray(class_sample_num_, dtype=np.float32)
    lab = np.asarray(label).astype(np.int64)

    safe = np.clip(norms, 0.001, 100.0)
    safe = safe / (csn[:, None] + 0.001)
    safe = np.clip(safe, 0.001, 100.0).astype(np.float32)
    mean = safe.mean(dtype=np.float64)
    std = safe.std(ddof=1, dtype=np.float64)
    ms = np.clip((safe.astype(np.float64) - mean) / (std + EPS) * H, -1.0, 1.0)[:, 0]

    rows = np.arange(B)
    emb64 = np.asarray(embbedings, dtype=np.float64)
    cols = np.asarray(kernel, dtype=np.float64)[:, lab]
    dots = np.einsum("be,eb->b", emb64, cols)
    c0 = np.clip(dots / np.linalg.norm(cols, axis=0), -1.0 + EPS, 1.0 - EPS)
    theta = np.arccos(c0) - MARGIN * ms
    theta = np.clip(theta, EPS, math.pi - EPS)
    val = (np.cos(theta) - (MARGIN + MARGIN * ms)) * S
    outT[lab, rows] = val.astype(np.float32)

    return np.ascontiguousarray(outT.T)
